# revision 19
# baseline (speedup 1.0000x reference)
"""Multi-head self-attention forward on 8 Trainium2 NeuronCores.

Problem: x[4,2048,512] -> qkv proj (w_qkv [512,1536]) -> 8-head attention
(head_dim 64) -> out proj (w_out [512,512] + b_out) -> y[4,2048,512].

Sharding: 8 shards = (batch b in 0..3) x (head-group hg in 0..1, 4 heads each).
Core c handles b=c//2, hg=c%2. Each core computes, for its batch and its 4
heads: qkv projection (only its heads' columns), attention, and the partial
output projection restricted to its heads' rows of w_out. Host sums the two
half-projections per batch and adds the bias.

On-device layout (all "T" tensors keep the contraction dim on partitions):
  xT   [512, 2048]   x[b] transposed (host-side transpose), one SBUF tile
       with the four 128-row chunks side by side in the free dim
  qkT  4 tiles [128, 2048]: Q01, K01, Q23, K23 (2 heads stacked per tile:
       head A on partitions 0:64, head B on 64:128)
  v_aug 16 seq-tiles [128, 4*65]: per head 64 v columns + a ones column
       (the ones column makes the oT matmul also produce the softmax
       denominator as row 64 of its output)
  sT   [k, q] scores transposed -> exp (no max subtraction: |s|~N(0,1), safe
       in fp32) -> pT
  oT   v_aug.T @ pT = [65, q]: rows 0:64 unnormalized head output (d on
       partitions), row 64 = softmax denominator

Out-projection (per block = one head-pair p, one 512-wide q chunk):
  reciprocal of the two denominator rows -> broadcast across 64 partitions
  with a K=33 selector matmul (R2) -> DVE-multiply into oT while casting to
  bf16 (oTs, normalized, heads A/B stacked on partitions) -> single K=128
  matmul per 128-q chunk against w2 covers both heads at once; p=0 result is
  copied to a SBUF accumulator, p=1 is added and DMA'd out. This replaces the
  K=64 matmul pairs + per-head tensor_scalar scaling + gpsimd adds of the
  earlier version (half the yproj PE rows, no transpose matmuls).
"""

import numpy as np

import concourse.bass as bass
import concourse.mybir as mybir
import concourse.tile as tile
from concourse import bacc

DIM = 512
NHEADS = 8
HD = 64
B = 4
SEQ = 2048
SCALE = HD ** -0.5

NCORES = 8
HPC = 4          # heads per core
QCH = 512        # q chunk (moving free dim)
NQC = SEQ // QCH # 4 q-chunks
KCH = 128        # k chunk (psum partition dim)
NKC = SEQ // KCH # 16 k-chunks
CCH = 128        # contraction chunk for projections
NCC = DIM // CCH # 4

F32 = mybir.dt.float32

BF16 = mybir.dt.bfloat16
# matmul input dtype. bf16: 1 cycle/row, FWL weight loads, half the PE power
# of f32r (less HAM throttling). fp8 was evaluated and fails the 2e-2
# correctness gate (rel err ~2.8e-2 in simulation).
MMDT = BF16


def _emit_o(nc, oA, oB, vaug_t, pt_pair, i, p, start, stop):
    """Accumulate the two kc chunks of pair-iteration i into oA/oB."""
    pA, pB = pt_pair
    for hh, (odst, psrc) in enumerate(((oA, pA), (oB, pB))):
        for half in range(2):
            kc = 2 * i + half
            nc.tensor.matmul(
                odst[:],
                vaug_t(kc)[:, 2 * p + hh, :],
                psrc[:, half * QCH:(half + 1) * QCH],
                start=(start and half == 0), stop=(stop and half == 1),
                skip_group_check=True,
            )


def build_nc():
    nc = bacc.Bacc()

    xT_d = nc.dram_tensor("xt", [DIM, SEQ], MMDT, kind="ExternalInput")
    wperm_d = nc.dram_tensor("wperm", [DIM, 4 * 128], MMDT, kind="ExternalInput")
    wv_d = nc.dram_tensor("wv", [DIM, HPC * HD], MMDT, kind="ExternalInput")
    w2_d = nc.dram_tensor("w2", [HPC * HD, DIM], MMDT, kind="ExternalInput")
    y_d = nc.dram_tensor("y", [SEQ, DIM], F32, kind="ExternalOutput")

    with tile.TileContext(nc) as tc:
        with (
            tc.tile_pool(name="const", bufs=1) as cpool,
            tc.tile_pool(name="big", bufs=1) as bigpool,
            tc.tile_pool(name="pt", bufs=4) as ptpool,
            tc.tile_pool(name="yacc", bufs=1) as yaccpool,
            tc.tile_pool(name="tmp", bufs=2) as tmppool,
            tc.tile_pool(name="small", bufs=2) as smallpool,
            tc.tile_pool(name="ps", bufs=1, space="PSUM") as ps,
        ):
            # ---- constants / inputs to SBUF ----
            xTt = cpool.tile([128, NCC * SEQ], MMDT, tag="xT", name="xT")
            wpt = cpool.tile([128, NCC * 512], MMDT, tag="wp", name="wp")
            wvt = cpool.tile([128, NCC * HPC * HD], MMDT, tag="wv", name="wv")
            w2t = cpool.tile([128, 2 * DIM], MMDT, tag="w2", name="w2")
            ones4 = cpool.tile([128, HPC], F32, tag="ones4")
            nc.gpsimd.memset(ones4[:], 1.0)
            ones1 = cpool.tile([1, 1], F32, tag="ones1")
            nc.gpsimd.memset(ones1[:], 1.0)
            # selector for the reciprocal broadcast: row 0 ones, rows 1:33
            # zero (K=33: K=1 matmuls fail an ISA check)
            selst = cpool.tile([33, 64], F32, tag="selst")
            nc.gpsimd.memset(selst[:], 0.0)
            nc.gpsimd.memset(selst[0:1, :], 1.0)
            sel64 = cpool.tile([33, 64], MMDT, tag="sel64")
            nc.vector.tensor_copy(sel64[:], selst[:])
            # preload the exp ACT table set early so the first real exp in
            # the attention phase doesn't stall the pipeline ~2.7us
            dummy = cpool.tile([1, 1], F32, tag="dummy")
            nc.scalar.activation(dummy[:], ones1[:],
                                 mybir.ActivationFunctionType.Exp)

            # three DMA-capable queues (sync/SP, scalar/ACT, gpsimd).
            # Plain 2D transfers (multi-dim interleaves start several us
            # late on hardware), ordered so the prelude's inputs (wperm
            # Q01/K01 cols, x cols 0:1024, wv) land first. Q23/K23 weight
            # cols and w2 are consumed tens of microseconds in.
            def xslice(c, a, b):
                return (xTt[:, c * SEQ + a:c * SEQ + b],
                        xT_d[c * 128:(c + 1) * 128, a:b])

            for c in range(NCC):   # Q01/K01 weight cols per c-chunk
                nc.sync.dma_start(wpt[:, c * 512:c * 512 + 256],
                                  wperm_d[c * 128:(c + 1) * 128, 0:256])
            for c in range(NCC):   # v weights per c-chunk
                nc.gpsimd.dma_start(
                    wvt[:, c * 256:(c + 1) * 256],
                    wv_d[c * 128:(c + 1) * 128, :])
            nc.scalar.dma_start(*xslice(0, 0, 1024))
            nc.scalar.dma_start(*xslice(1, 0, 1024))
            nc.gpsimd.dma_start(*xslice(2, 0, 1024))
            nc.sync.dma_start(*xslice(3, 0, 1024))
            nc.scalar.dma_start(*xslice(2, 1024, SEQ))
            nc.gpsimd.dma_start(*xslice(0, 1024, SEQ))
            nc.sync.dma_start(*xslice(3, 1024, SEQ))
            nc.scalar.dma_start(*xslice(1, 1024, SEQ))
            for c in range(NCC):   # Q23/K23 weight cols
                nc.sync.dma_start(wpt[:, c * 512 + 256:(c + 1) * 512],
                                  wperm_d[c * 128:(c + 1) * 128, 256:512])
            for g in range(2):
                nc.gpsimd.dma_start(w2t[:, g * DIM:(g + 1) * DIM],
                                    w2_d[g * 128:(g + 1) * 128, :])

            def xT_c(c):
                return xTt[:, c * SEQ:(c + 1) * SEQ]

            def wp_c(c):
                return wpt[:, c * 512:(c + 1) * 512]

            def wv_c(c):
                return wvt[:, c * (HPC * HD):(c + 1) * (HPC * HD)]

            def w2_p(p):
                return w2t[:, p * DIM:(p + 1) * DIM]

            # ---- persistent intermediates ----
            qkTs = [bigpool.tile([128, SEQ], MMDT, tag=f"qkT{m}",
                                 name=f"qkT{m}") for m in range(4)]
            vaugs = [bigpool.tile([128, HPC * 65], MMDT, tag=f"vaug{st}",
                                  name=f"vaug{st}") for st in range(NKC)]
            yacc = yaccpool.tile([128, SEQ // 128 * DIM], F32, tag="yacc")

            # zero-init the rcp33 pool bufs once; later writes touch row 0
            # only, so rows 1:33 stay zero for the K=33 broadcast matmul
            for _ in range(2):
                t = smallpool.tile([33, 2 * QCH], MMDT, tag="rcp33")
                nc.gpsimd.memset(t[:], 0.0)

            def qkT_blk(m):
                return qkTs[m]

            def vaug_t(kc):
                return vaugs[kc].rearrange("p (h e) -> p h e", e=65)

            def v_unit(st, tag, bufs):
                pv = ps.tile([128, HPC * HD], F32, tag=tag, bufs=bufs,
                             name="pv")
                for c in range(NCC):
                    nc.tensor.matmul(
                        pv[:],
                        xT_c(c)[:, st * 128:(st + 1) * 128],
                        wv_c(c)[:],
                        start=(c == 0), stop=(c == NCC - 1),
                        skip_group_check=True,
                    )
                vt = vaug_t(st)
                nc.vector.tensor_copy(
                    vt[:, :, 0:64], pv[:].rearrange("p (h d) -> p h d", d=HD)
                )
                nc.vector.tensor_copy(
                    vt[:, :, 64:65],
                    ones4[:].rearrange("p (h o) -> p h o", o=1))

            def qk_unit(m, s2, tag, bufs):
                pp = ps.tile([128, 512], F32, tag=tag, bufs=bufs, name="pp")
                for c in range(NCC):
                    nc.tensor.matmul(
                        pp[:],
                        wp_c(c)[:, m * 128:(m + 1) * 128],
                        xT_c(c)[:, s2 * 512:(s2 + 1) * 512],
                        start=(c == 0), stop=(c == NCC - 1),
                        skip_group_check=True,
                    )
                nc.vector.tensor_copy(qkTs[m][:, s2 * 512:(s2 + 1) * 512],
                                      pp[:])

            # ---- phase 1: minimal prelude, rest is in-block filler ----
            # The first attention block needs only Q01 for its q-chunk
            # (cols 0:512), K01 for its first k-chunks, and the first two
            # v seq-tiles; everything else is computed as filler inside
            # the attention blocks, scheduled against its first use.
            qk_unit(0, 0, "sA", 1)   # Q01 q 0:512
            qk_unit(1, 0, "sB", 1)   # K01 k 0:512   (kc 0..3)
            qk_unit(1, 1, "sA", 1)   # K01 k 512:1024 (kc 4..7)
            v_unit(0, "sB", 1)
            v_unit(1, "y", 2)

            def V(st):
                return lambda: v_unit(st, "y", 2)

            def QK(m, s2):
                return lambda: qk_unit(m, s2, "y", 2)

            # filler schedule keyed by (p, qc, i). Deadlines: vaug(k) is
            # consumed by the o-emit at iteration k//2+1 of EVERY block (so
            # all v by end of block (0,0)); K01 s2=k at iteration 2k of
            # (0,0); Q01 s2=k at block (0,k); K23 all at (1,0); Q23 s2=k at
            # block (1,k).
            SCHED = {
                (0, 0, 0): [V(2), V(3)],
                (0, 0, 1): [V(4), V(5), QK(1, 2)],
                (0, 0, 2): [V(6), V(7)],
                (0, 0, 3): [V(8), V(9), QK(1, 3)],
                (0, 0, 4): [V(10), V(11)],
                (0, 0, 5): [V(12), V(13)],
                (0, 0, 6): [V(14), V(15), QK(0, 1)],
                (0, 1, 1): [QK(3, 0)],
                (0, 1, 3): [QK(0, 2)],
                (0, 1, 5): [QK(3, 1)],
                (0, 1, 7): [QK(3, 2)],
                (0, 2, 1): [QK(0, 3)],
                (0, 2, 3): [QK(3, 3)],
                (0, 2, 5): [QK(2, 0)],
                (0, 3, 1): [QK(2, 1)],
                (1, 0, 1): [QK(2, 2)],
                (1, 1, 1): [QK(2, 3)],
            }

            # ---- out-projection helpers ----
            def emit_release(p, qc, oA, oB):
                """Block epilogue: read oA/oB out quickly (reciprocal of the
                denominator rows + bf16 copy of the head outputs) so the
                next block's o-accumulation isn't WAR-stalled on them."""
                otu = tmppool.tile([128, QCH], MMDT, tag="otu")
                rcps = smallpool.tile([1, 2 * QCH], F32, tag="rcps")
                rcp33 = smallpool.tile([33, 2 * QCH], MMDT, tag="rcp33")
                # approx reciprocal (~18 bits, denominators are 50..104 so
                # no edge cases) is ~5x faster than DVE reciprocal(); the
                # exact one (4us per call) stalled the whole release chain.
                # stage the denominator rows to SBUF first (custom-DVE op
                # misbehaves on a PSUM source).
                dens = smallpool.tile([1, 2 * QCH], F32, tag="dens")
                nc.vector.tensor_copy(dens[:, 0:QCH], oA[64:65, :])
                nc.vector.tensor_copy(dens[:, QCH:2 * QCH], oB[64:65, :])
                nc.vector.reciprocal_approx_fast(rcps[:], dens[:])
                nc.vector.tensor_copy(otu[0:64, :], oA[0:64, :])
                nc.vector.tensor_copy(otu[64:128, :], oB[0:64, :])
                nc.vector.tensor_copy(rcp33[0:1, :], rcps[:])
                return {"p": p, "qc": qc, "otu": otu, "rcp33": rcp33}

            def emit_R2_mults(pend):
                """Broadcast the reciprocals across 64 partitions (K=33
                selector matmuls) and scale otu into the normalized bf16
                stationary tile for the out-projection."""
                rcp33, otu = pend["rcp33"], pend["otu"]
                R2a = ps.tile([64, QCH], F32, tag="y", bufs=2, name="R2a")
                nc.tensor.matmul(R2a[:], sel64[:], rcp33[:, 0:QCH],
                                 start=True, stop=True, skip_group_check=True)
                R2b = ps.tile([64, QCH], F32, tag="y", bufs=2, name="R2b")
                nc.tensor.matmul(R2b[:], sel64[:], rcp33[:, QCH:2 * QCH],
                                 start=True, stop=True, skip_group_check=True)
                ot = tmppool.tile([128, QCH], MMDT, tag="oTs")
                nc.vector.tensor_mul(ot[0:64, :], otu[0:64, :], R2a[:])
                nc.vector.tensor_mul(ot[64:128, :], otu[64:128, :], R2b[:])
                pend["ot"] = ot

            def emit_yproj_j(pend, j):
                p, qc, ot = pend["p"], pend["qc"], pend["ot"]
                qt = qc * (QCH // 128) + j
                yps = ps.tile([128, DIM], F32, tag="y", bufs=2, name="yps")
                nc.tensor.matmul(
                    yps[:],
                    ot[:, j * 128:(j + 1) * 128],
                    w2_p(p)[:],
                    start=True, stop=True, skip_group_check=True,
                )
                ya = yacc[:, qt * DIM:(qt + 1) * DIM]
                if p == 0:
                    nc.vector.tensor_copy(ya, yps[:])
                else:
                    nc.vector.tensor_add(ya, ya, yps[:])
                    nc.sync.dma_start(y_d[qt * 128:(qt + 1) * 128, :], ya)

            # ---- phase 2: attention + out-proj ----
            # kc chunks processed in pairs: one s psum tile [128, 1024] holds
            # scores for kc and kc+1 side by side, halving ACT instruction
            # count. Two levels of software pipelining keep the PE stream
            # dense: within a block, s(i+1) is emitted before o(i) so the PE
            # never head-of-line blocks on exp(i); across blocks, the
            # out-projection of block n is spread into the first
            # pair-iterations of block n+1.
            NPAIR = NKC // 2

            pending = None
            for p in range(2):
                Q = qkT_blk(2 * p)
                K = qkT_blk(2 * p + 1)
                for qc in range(NQC):
                    oA = ps.tile([65, QCH], F32, tag="oA", bufs=1, name="oA")
                    oB = ps.tile([65, QCH], F32, tag="oB", bufs=1, name="oB")
                    prev = None
                    for i in range(NPAIR):
                        sA = ps.tile([128, 2 * QCH], F32, tag="sA", bufs=1,
                                     name="sA")
                        sB = ps.tile([128, 2 * QCH], F32, tag="sB", bufs=1,
                                     name="sB")
                        for hh, stile in ((0, sA), (1, sB)):
                            for half in range(2):
                                kc = 2 * i + half
                                nc.tensor.matmul(
                                    stile[:, half * QCH:(half + 1) * QCH],
                                    K[64 * hh:64 * hh + 64,
                                      kc * 128:(kc + 1) * 128],
                                    Q[64 * hh:64 * hh + 64,
                                      qc * QCH:(qc + 1) * QCH],
                                    start=True, stop=True,
                                    skip_group_check=True,
                                )
                        for f in SCHED.get((p, qc, i), ()):
                            f()
                        # previous block's out-projection, spread across
                        # this block's pair-iterations: the R2/scale stage
                        # at i=2 (after enough PE work that the PE doesn't
                        # head-of-line block on the DVE chain), the four
                        # K=128 projection matmuls at i=4..7
                        if pending is not None:
                            if i == 2:
                                emit_R2_mults(pending)
                            elif 4 <= i < 4 + QCH // 128:
                                emit_yproj_j(pending, i - 4)
                                if i == 3 + QCH // 128:
                                    pending = None
                        if prev is not None:
                            _emit_o(nc, oA, oB, vaug_t, prev, i - 1, p,
                                    start=(i == 1), stop=(i == NPAIR - 1))
                        pA = ptpool.tile([128, 2 * QCH], MMDT, tag="pA")
                        pB = ptpool.tile([128, 2 * QCH], MMDT, tag="pB")
                        nc.scalar.activation(
                            pA[:], sA[:], mybir.ActivationFunctionType.Exp,
                            scale=SCALE,
                        )
                        nc.scalar.activation(
                            pB[:], sB[:], mybir.ActivationFunctionType.Exp,
                            scale=SCALE,
                        )
                        prev = (pA, pB)
                    _emit_o(nc, oA, oB, vaug_t, prev, NPAIR - 1, p,
                            start=(NPAIR == 1), stop=True)
                    pending = emit_release(p, qc, oA, oB)

            # tail: last block's out-projection
            emit_R2_mults(pending)
            for j in range(QCH // 128):
                emit_yproj_j(pending, j)

    nc.finalize()
    return nc


_NC_CACHE = {}


def get_nc():
    if "nc" not in _NC_CACHE:
        _NC_CACHE["nc"] = build_nc()
    return _NC_CACHE["nc"]


def make_core_inputs(x, w_qkv, w_out):
    """Per-core input dicts (host-side sharding)."""
    in_maps = []
    for c in range(NCORES):
        b, hg = c // 2, c % 2
        heads = [hg * HPC + i for i in range(HPC)]
        qcols = [w_qkv[:, h * HD:(h + 1) * HD] for h in heads]
        kcols = [w_qkv[:, DIM + h * HD:DIM + (h + 1) * HD] for h in heads]
        vcols = [w_qkv[:, 2 * DIM + h * HD:2 * DIM + (h + 1) * HD] for h in heads]
        wperm = np.concatenate(
            [qcols[0], qcols[1], kcols[0], kcols[1],
             qcols[2], qcols[3], kcols[2], kcols[3]], axis=1)
        wv = np.concatenate(vcols, axis=1)
        w2 = w_out[hg * HPC * HD:(hg + 1) * HPC * HD, :]
        import ml_dtypes
        mmnp = (ml_dtypes.bfloat16 if MMDT == mybir.dt.bfloat16
                else np.float32)
        in_maps.append({
            "xt": np.ascontiguousarray(x[b].T).astype(mmnp),
            "wperm": np.ascontiguousarray(wperm).astype(mmnp),
            "wv": np.ascontiguousarray(wv).astype(mmnp),
            "w2": np.ascontiguousarray(w2).astype(mmnp),
        })
    return in_maps


def kernel(x, w_qkv, w_out, b_out):
    from concourse.bass_utils import run_bass_kernel_spmd

    x = np.asarray(x, dtype=np.float32)
    w_qkv = np.asarray(w_qkv, dtype=np.float32)
    w_out = np.asarray(w_out, dtype=np.float32)
    b_out = np.asarray(b_out, dtype=np.float32)

    nc = get_nc()
    in_maps = make_core_inputs(x, w_qkv, w_out)
    res = run_bass_kernel_spmd(nc, in_maps, list(range(NCORES))).results

    out = np.empty((B, SEQ, DIM), dtype=np.float32)
    for b in range(B):
        out[b] = res[2 * b]["y"] + res[2 * b + 1]["y"] + b_out
    return out


# revision 23
# speedup vs baseline: 1.0080x; 1.0080x over previous
"""Multi-head self-attention forward on 8 Trainium2 NeuronCores.

Problem: x[4,2048,512] -> qkv proj (w_qkv [512,1536]) -> 8-head attention
(head_dim 64) -> out proj (w_out [512,512] + b_out) -> y[4,2048,512].

Sharding: 8 shards = (batch b in 0..3) x (head-group hg in 0..1, 4 heads each).
Core c handles b=c//2, hg=c%2. Each core computes, for its batch and its 4
heads: qkv projection (only its heads' columns), attention, and the partial
output projection restricted to its heads' rows of w_out. Host sums the two
half-projections per batch and adds the bias.

On-device layout (all "T" tensors keep the contraction dim on partitions):
  xT   [512, 2048]   x[b] transposed (host-side transpose), one SBUF tile
       with the four 128-row chunks side by side in the free dim
  qkT  4 tiles [128, 2048]: Q01, K01, Q23, K23 (2 heads stacked per tile:
       head A on partitions 0:64, head B on 64:128)
  v_aug 16 seq-tiles [128, 4*65]: per head 64 v columns + a ones column
       (the ones column makes the oT matmul also produce the softmax
       denominator as row 64 of its output)
  sT   [k, q] scores transposed -> exp (no max subtraction: |s|~N(0,1), safe
       in fp32) -> pT
  oT   v_aug.T @ pT = [65, q]: rows 0:64 unnormalized head output (d on
       partitions), row 64 = softmax denominator

Out-projection (per block = one head-pair p, one 512-wide q chunk):
  reciprocal of the two denominator rows -> broadcast across 64 partitions
  with a K=33 selector matmul (R2) -> DVE-multiply into oT while casting to
  bf16 (oTs, normalized, heads A/B stacked on partitions) -> single K=128
  matmul per 128-q chunk against w2 covers both heads at once; p=0 result is
  copied to a SBUF accumulator, p=1 is added and DMA'd out. This replaces the
  K=64 matmul pairs + per-head tensor_scalar scaling + gpsimd adds of the
  earlier version (half the yproj PE rows, no transpose matmuls).
"""

import numpy as np

import concourse.bass as bass
import concourse.mybir as mybir
import concourse.tile as tile
from concourse import bacc

DIM = 512
NHEADS = 8
HD = 64
B = 4
SEQ = 2048
SCALE = HD ** -0.5

NCORES = 8
HPC = 4          # heads per core
QCH = 512        # q chunk (moving free dim)
NQC = SEQ // QCH # 4 q-chunks
KCH = 128        # k chunk (psum partition dim)
NKC = SEQ // KCH # 16 k-chunks
CCH = 128        # contraction chunk for projections
NCC = DIM // CCH # 4

F32 = mybir.dt.float32

BF16 = mybir.dt.bfloat16
# matmul input dtype. bf16: 1 cycle/row, FWL weight loads, half the PE power
# of f32r (less HAM throttling). fp8 was evaluated and fails the 2e-2
# correctness gate (rel err ~2.8e-2 in simulation).
MMDT = BF16


def _emit_o(nc, oA, oB, vaug_t, pt_pair, i, p, start, stop):
    """Accumulate the two kc chunks of pair-iteration i into oA/oB."""
    pA, pB = pt_pair
    for hh, (odst, psrc) in enumerate(((oA, pA), (oB, pB))):
        for half in range(2):
            kc = 2 * i + half
            nc.tensor.matmul(
                odst[:],
                vaug_t(kc)[:, 2 * p + hh, :],
                psrc[:, half * QCH:(half + 1) * QCH],
                start=(start and half == 0), stop=(stop and half == 1),
                skip_group_check=True,
            )


def build_nc():
    nc = bacc.Bacc()

    xT_d = nc.dram_tensor("xt", [DIM, SEQ], MMDT, kind="ExternalInput")
    wperm_d = nc.dram_tensor("wperm", [DIM, 4 * 128], MMDT, kind="ExternalInput")
    wv_d = nc.dram_tensor("wv", [DIM, HPC * HD], MMDT, kind="ExternalInput")
    w2_d = nc.dram_tensor("w2", [HPC * HD, DIM], MMDT, kind="ExternalInput")
    y_d = nc.dram_tensor("y", [SEQ, DIM], F32, kind="ExternalOutput")

    with tile.TileContext(nc) as tc:
        with (
            tc.tile_pool(name="const", bufs=1) as cpool,
            tc.tile_pool(name="big", bufs=1) as bigpool,
            tc.tile_pool(name="pt", bufs=4) as ptpool,
            tc.tile_pool(name="yacc", bufs=1) as yaccpool,
            tc.tile_pool(name="tmp", bufs=2) as tmppool,
            tc.tile_pool(name="small", bufs=2) as smallpool,
            tc.tile_pool(name="ps", bufs=1, space="PSUM") as ps,
        ):
            # ---- constants / inputs to SBUF ----
            xTt = cpool.tile([128, NCC * SEQ], MMDT, tag="xT", name="xT")
            wpt = cpool.tile([128, NCC * 512], MMDT, tag="wp", name="wp")
            wvt = cpool.tile([128, NCC * HPC * HD], MMDT, tag="wv", name="wv")
            w2t = cpool.tile([128, 2 * DIM], MMDT, tag="w2", name="w2")
            ones4 = cpool.tile([128, HPC], F32, tag="ones4")
            nc.gpsimd.memset(ones4[:], 1.0)
            ones1 = cpool.tile([1, 1], F32, tag="ones1")
            nc.gpsimd.memset(ones1[:], 1.0)
            # selector for the reciprocal broadcast: row 0 ones, rows 1:33
            # zero (K=33: K=1 matmuls fail an ISA check)
            selst = cpool.tile([33, 64], F32, tag="selst")
            nc.gpsimd.memset(selst[:], 0.0)
            nc.gpsimd.memset(selst[0:1, :], 1.0)
            sel64 = cpool.tile([33, 64], MMDT, tag="sel64")
            nc.vector.tensor_copy(sel64[:], selst[:])
            # preload the exp ACT table set early so the first real exp in
            # the attention phase doesn't stall the pipeline ~2.7us
            dummy = cpool.tile([1, 1], F32, tag="dummy")
            nc.scalar.activation(dummy[:], ones1[:],
                                 mybir.ActivationFunctionType.Exp)

            # three DMA-capable queues (sync/SP, scalar/ACT, gpsimd).
            # Plain 2D transfers (multi-dim interleaves start several us
            # late on hardware), ordered so the prelude's inputs (wperm
            # Q01/K01 cols, x cols 0:1024, wv) land first. Q23/K23 weight
            # cols and w2 are consumed tens of microseconds in.
            def xslice(c, a, b):
                return (xTt[:, c * SEQ + a:c * SEQ + b],
                        xT_d[c * 128:(c + 1) * 128, a:b])

            for c in range(NCC):   # Q01/K01 weight cols per c-chunk
                nc.sync.dma_start(wpt[:, c * 512:c * 512 + 256],
                                  wperm_d[c * 128:(c + 1) * 128, 0:256])
            for c in range(NCC):   # v weights per c-chunk
                nc.gpsimd.dma_start(
                    wvt[:, c * 256:(c + 1) * 256],
                    wv_d[c * 128:(c + 1) * 128, :])
            # x head halves, one per queue + tails balanced ~1MB/queue
            nc.scalar.dma_start(*xslice(0, 0, 1024))
            nc.sync.dma_start(*xslice(3, 0, 1024))
            nc.gpsimd.dma_start(*xslice(2, 0, 1024))
            nc.scalar.dma_start(*xslice(1, 0, 1024))
            nc.gpsimd.dma_start(*xslice(2, 1024, SEQ))
            nc.sync.dma_start(*xslice(3, 1024, SEQ))
            nc.scalar.dma_start(*xslice(1, 1024, SEQ))
            nc.scalar.dma_start(*xslice(0, 1024, SEQ))
            for c in range(NCC):   # Q23/K23 weight cols
                nc.sync.dma_start(wpt[:, c * 512 + 256:(c + 1) * 512],
                                  wperm_d[c * 128:(c + 1) * 128, 256:512])
            for g in range(2):
                nc.gpsimd.dma_start(w2t[:, g * DIM:(g + 1) * DIM],
                                    w2_d[g * 128:(g + 1) * 128, :])

            def xT_c(c):
                return xTt[:, c * SEQ:(c + 1) * SEQ]

            def wp_c(c):
                return wpt[:, c * 512:(c + 1) * 512]

            def wv_c(c):
                return wvt[:, c * (HPC * HD):(c + 1) * (HPC * HD)]

            def w2_p(p):
                return w2t[:, p * DIM:(p + 1) * DIM]

            # ---- persistent intermediates ----
            qkTs = [bigpool.tile([128, SEQ], MMDT, tag=f"qkT{m}",
                                 name=f"qkT{m}") for m in range(4)]
            vaugs = [bigpool.tile([128, HPC * 65], MMDT, tag=f"vaug{st}",
                                  name=f"vaug{st}") for st in range(NKC)]
            yacc = yaccpool.tile([128, SEQ // 128 * DIM], F32, tag="yacc")

            # zero-init the rcp33 pool bufs once; later writes touch row 0
            # only, so rows 1:33 stay zero for the K=33 broadcast matmul
            for _ in range(2):
                t = smallpool.tile([33, 2 * QCH], MMDT, tag="rcp33")
                nc.gpsimd.memset(t[:], 0.0)

            def qkT_blk(m):
                return qkTs[m]

            def vaug_t(kc):
                return vaugs[kc].rearrange("p (h e) -> p h e", e=65)

            def v_unit(st, tag, bufs):
                pv = ps.tile([128, HPC * HD], F32, tag=tag, bufs=bufs,
                             name="pv")
                for c in range(NCC):
                    nc.tensor.matmul(
                        pv[:],
                        xT_c(c)[:, st * 128:(st + 1) * 128],
                        wv_c(c)[:],
                        start=(c == 0), stop=(c == NCC - 1),
                        skip_group_check=True,
                    )
                vt = vaug_t(st)
                nc.vector.tensor_copy(
                    vt[:, :, 0:64], pv[:].rearrange("p (h d) -> p h d", d=HD)
                )
                nc.vector.tensor_copy(
                    vt[:, :, 64:65],
                    ones4[:].rearrange("p (h o) -> p h o", o=1))

            def qk_unit(m, s2, tag, bufs):
                pp = ps.tile([128, 512], F32, tag=tag, bufs=bufs, name="pp")
                for c in range(NCC):
                    nc.tensor.matmul(
                        pp[:],
                        wp_c(c)[:, m * 128:(m + 1) * 128],
                        xT_c(c)[:, s2 * 512:(s2 + 1) * 512],
                        start=(c == 0), stop=(c == NCC - 1),
                        skip_group_check=True,
                    )
                nc.vector.tensor_copy(qkTs[m][:, s2 * 512:(s2 + 1) * 512],
                                      pp[:])

            # ---- phase 1: minimal prelude, rest is in-block filler ----
            # The first attention block needs only Q01 for its q-chunk
            # (cols 0:512), K01 for its first k-chunks, and the first two
            # v seq-tiles; everything else is computed as filler inside
            # the attention blocks, scheduled against its first use.
            qk_unit(0, 0, "sA", 1)   # Q01 q 0:512
            qk_unit(1, 0, "sB", 1)   # K01 k 0:512   (kc 0..3)
            qk_unit(1, 1, "sA", 1)   # K01 k 512:1024 (kc 4..7)
            v_unit(0, "sB", 1)
            v_unit(1, "y", 2)

            def V(st):
                return lambda: v_unit(st, "y", 2)

            def QK(m, s2):
                return lambda: qk_unit(m, s2, "y", 2)

            # filler schedule keyed by (p, qc, i). Deadlines: vaug(k) is
            # consumed by the o-emit at iteration k//2+1 of EVERY block (so
            # all v by end of block (0,0)); K01 s2=k at iteration 2k of
            # (0,0); Q01 s2=k at block (0,k); K23 all at (1,0); Q23 s2=k at
            # block (1,k).
            SCHED = {
                (0, 0, 0): [V(2), V(3)],
                (0, 0, 1): [V(4), V(5), QK(1, 2)],
                (0, 0, 2): [V(6), V(7)],
                (0, 0, 3): [V(8), V(9), QK(1, 3)],
                (0, 0, 4): [V(10), V(11)],
                (0, 0, 5): [V(12), V(13)],
                (0, 0, 6): [V(14), V(15), QK(0, 1)],
                (0, 1, 1): [QK(3, 0)],
                (0, 1, 3): [QK(0, 2)],
                (0, 1, 5): [QK(3, 1)],
                (0, 1, 7): [QK(3, 2)],
                (0, 2, 1): [QK(0, 3)],
                (0, 2, 3): [QK(3, 3)],
                (0, 2, 5): [QK(2, 0)],
                (0, 3, 1): [QK(2, 1)],
                (1, 0, 1): [QK(2, 2)],
                (1, 1, 1): [QK(2, 3)],
            }

            # ---- out-projection helpers ----
            def emit_release(p, qc, oA, oB):
                """Block epilogue: read oA/oB out quickly (reciprocal of the
                denominator rows + bf16 copy of the head outputs) so the
                next block's o-accumulation isn't WAR-stalled on them."""
                # single [0:65] copy per psum tile (head rows + denominator
                # row together) frees oA/oB for the next block's o-matmuls
                # ASAP; the otu/den splits then read the SBUF stage, with
                # the bulk otu copies on the otherwise-idle Pool engine
                stA = tmppool.tile([65, QCH], F32, tag="stA")
                stB = tmppool.tile([65, QCH], F32, tag="stB")
                nc.vector.tensor_copy(stA[:], oA[:])
                nc.vector.tensor_copy(stB[:], oB[:])
                otu = tmppool.tile([128, QCH], MMDT, tag="otu")
                nc.gpsimd.tensor_copy(otu[0:64, :], stA[0:64, :])
                nc.gpsimd.tensor_copy(otu[64:128, :], stB[0:64, :])
                dens = smallpool.tile([1, 2 * QCH], F32, tag="dens")
                nc.vector.tensor_copy(dens[:, 0:QCH], stA[64:65, :])
                nc.vector.tensor_copy(dens[:, QCH:2 * QCH], stB[64:65, :])
                # approx reciprocal (~18 bits, denominators are 50..104 so
                # no edge cases) is ~5x faster than DVE reciprocal(); the
                # exact one (4us per call) stalled the whole release chain.
                rcps = smallpool.tile([1, 2 * QCH], F32, tag="rcps")
                rcp33 = smallpool.tile([33, 2 * QCH], MMDT, tag="rcp33")
                nc.vector.reciprocal_approx_fast(rcps[:], dens[:])
                nc.vector.tensor_copy(rcp33[0:1, :], rcps[:])
                return {"p": p, "qc": qc, "otu": otu, "rcp33": rcp33}

            def emit_R2_mults(pend):
                """Broadcast the reciprocals across 64 partitions (K=33
                selector matmuls) and scale otu into the normalized bf16
                stationary tile for the out-projection."""
                rcp33, otu = pend["rcp33"], pend["otu"]
                R2a = ps.tile([64, QCH], F32, tag="y", bufs=2, name="R2a")
                nc.tensor.matmul(R2a[:], sel64[:], rcp33[:, 0:QCH],
                                 start=True, stop=True, skip_group_check=True)
                R2b = ps.tile([64, QCH], F32, tag="y", bufs=2, name="R2b")
                nc.tensor.matmul(R2b[:], sel64[:], rcp33[:, QCH:2 * QCH],
                                 start=True, stop=True, skip_group_check=True)
                ot = tmppool.tile([128, QCH], MMDT, tag="oTs")
                nc.vector.tensor_mul(ot[0:64, :], otu[0:64, :], R2a[:])
                nc.vector.tensor_mul(ot[64:128, :], otu[64:128, :], R2b[:])
                pend["ot"] = ot

            def emit_yproj_j(pend, j):
                p, qc, ot = pend["p"], pend["qc"], pend["ot"]
                qt = qc * (QCH // 128) + j
                yps = ps.tile([128, DIM], F32, tag="y", bufs=2, name="yps")
                nc.tensor.matmul(
                    yps[:],
                    ot[:, j * 128:(j + 1) * 128],
                    w2_p(p)[:],
                    start=True, stop=True, skip_group_check=True,
                )
                ya = yacc[:, qt * DIM:(qt + 1) * DIM]
                if p == 0:
                    nc.vector.tensor_copy(ya, yps[:])
                else:
                    nc.vector.tensor_add(ya, ya, yps[:])
                    nc.sync.dma_start(y_d[qt * 128:(qt + 1) * 128, :], ya)

            # ---- phase 2: attention + out-proj ----
            # kc chunks processed in pairs: one s psum tile [128, 1024] holds
            # scores for kc and kc+1 side by side, halving ACT instruction
            # count. Two levels of software pipelining keep the PE stream
            # dense: within a block, s(i+1) is emitted before o(i) so the PE
            # never head-of-line blocks on exp(i); across blocks, the
            # out-projection of block n is spread into the first
            # pair-iterations of block n+1.
            NPAIR = NKC // 2

            pending = None
            for p in range(2):
                Q = qkT_blk(2 * p)
                K = qkT_blk(2 * p + 1)
                for qc in range(NQC):
                    oA = ps.tile([65, QCH], F32, tag="oA", bufs=1, name="oA")
                    oB = ps.tile([65, QCH], F32, tag="oB", bufs=1, name="oB")
                    # o-emission lags the scores by 1 pair-iteration, except
                    # that the first two pairs both emit at i=2: the first
                    # write to oA/oB then comes ~6.6us after the previous
                    # block ended, giving the release copies a wide margin
                    # (a WAR stall here demotes the PE clock for ~10us)
                    pend_o = []

                    def drain_o(upto, stop=False):
                        while pend_o and pend_o[0][0] <= upto:
                            j, pp_ = pend_o.pop(0)
                            _emit_o(nc, oA, oB, vaug_t, pp_, j, p,
                                    start=(j == 0),
                                    stop=(stop and j == NPAIR - 1))

                    for i in range(NPAIR):
                        sA = ps.tile([128, 2 * QCH], F32, tag="sA", bufs=1,
                                     name="sA")
                        sB = ps.tile([128, 2 * QCH], F32, tag="sB", bufs=1,
                                     name="sB")
                        for hh, stile in ((0, sA), (1, sB)):
                            for half in range(2):
                                kc = 2 * i + half
                                nc.tensor.matmul(
                                    stile[:, half * QCH:(half + 1) * QCH],
                                    K[64 * hh:64 * hh + 64,
                                      kc * 128:(kc + 1) * 128],
                                    Q[64 * hh:64 * hh + 64,
                                      qc * QCH:(qc + 1) * QCH],
                                    start=True, stop=True,
                                    skip_group_check=True,
                                )
                        for f in SCHED.get((p, qc, i), ()):
                            f()
                        # previous block's out-projection, spread across
                        # this block's pair-iterations: the R2/scale stage
                        # at i=2 (after enough PE work that the PE doesn't
                        # head-of-line block on the DVE chain), the four
                        # K=128 projection matmuls at i=4..7
                        if pending is not None:
                            if i == 2:
                                emit_R2_mults(pending)
                            elif 4 <= i < 4 + QCH // 128:
                                emit_yproj_j(pending, i - 4)
                                if i == 3 + QCH // 128:
                                    pending = None
                        if i >= 2:
                            drain_o(i - 1)
                        pA = ptpool.tile([128, 2 * QCH], MMDT, tag="pA")
                        pB = ptpool.tile([128, 2 * QCH], MMDT, tag="pB")
                        nc.scalar.activation(
                            pA[:], sA[:], mybir.ActivationFunctionType.Exp,
                            scale=SCALE,
                        )
                        nc.scalar.activation(
                            pB[:], sB[:], mybir.ActivationFunctionType.Exp,
                            scale=SCALE,
                        )
                        pend_o.append((i, (pA, pB)))
                    drain_o(NPAIR - 1, stop=True)
                    pending = emit_release(p, qc, oA, oB)

            # tail: last block's out-projection
            emit_R2_mults(pending)
            for j in range(QCH // 128):
                emit_yproj_j(pending, j)

    nc.finalize()
    return nc


_NC_CACHE = {}


def get_nc():
    if "nc" not in _NC_CACHE:
        _NC_CACHE["nc"] = build_nc()
    return _NC_CACHE["nc"]


def make_core_inputs(x, w_qkv, w_out):
    """Per-core input dicts (host-side sharding)."""
    in_maps = []
    for c in range(NCORES):
        b, hg = c // 2, c % 2
        heads = [hg * HPC + i for i in range(HPC)]
        qcols = [w_qkv[:, h * HD:(h + 1) * HD] for h in heads]
        kcols = [w_qkv[:, DIM + h * HD:DIM + (h + 1) * HD] for h in heads]
        vcols = [w_qkv[:, 2 * DIM + h * HD:2 * DIM + (h + 1) * HD] for h in heads]
        wperm = np.concatenate(
            [qcols[0], qcols[1], kcols[0], kcols[1],
             qcols[2], qcols[3], kcols[2], kcols[3]], axis=1)
        wv = np.concatenate(vcols, axis=1)
        w2 = w_out[hg * HPC * HD:(hg + 1) * HPC * HD, :]
        import ml_dtypes
        mmnp = (ml_dtypes.bfloat16 if MMDT == mybir.dt.bfloat16
                else np.float32)
        in_maps.append({
            "xt": np.ascontiguousarray(x[b].T).astype(mmnp),
            "wperm": np.ascontiguousarray(wperm).astype(mmnp),
            "wv": np.ascontiguousarray(wv).astype(mmnp),
            "w2": np.ascontiguousarray(w2).astype(mmnp),
        })
    return in_maps


def kernel(x, w_qkv, w_out, b_out):
    from concourse.bass_utils import run_bass_kernel_spmd

    x = np.asarray(x, dtype=np.float32)
    w_qkv = np.asarray(w_qkv, dtype=np.float32)
    w_out = np.asarray(w_out, dtype=np.float32)
    b_out = np.asarray(b_out, dtype=np.float32)

    nc = get_nc()
    in_maps = make_core_inputs(x, w_qkv, w_out)
    res = run_bass_kernel_spmd(nc, in_maps, list(range(NCORES))).results

    out = np.empty((B, SEQ, DIM), dtype=np.float32)
    for b in range(B):
        out[b] = res[2 * b]["y"] + res[2 * b + 1]["y"] + b_out
    return out


# revision 34
# speedup vs baseline: 1.0517x; 1.0434x over previous
"""Multi-head self-attention forward on 8 Trainium2 NeuronCores.

Problem: x[4,2048,512] -> qkv proj (w_qkv [512,1536]) -> 8-head attention
(head_dim 64) -> out proj (w_out [512,512] + b_out) -> y[4,2048,512].

Sharding: 8 shards = (batch b in 0..3) x (head-group hg in 0..1, 4 heads each).
Core c handles b=c//2, hg=c%2. Each core computes, for its batch and its 4
heads: qkv projection (only its heads' columns), attention, and the partial
output projection restricted to its heads' rows of w_out. Host sums the two
half-projections per batch and adds the bias.

On-device layout (all "T" tensors keep the contraction dim on partitions):
  xT   [512, 2048]   x[b] transposed (host-side transpose), one SBUF tile
       with the four 128-row chunks side by side in the free dim
  qkT  4 tiles [128, 2048]: Q01, K01, Q23, K23 (2 heads stacked per tile:
       head A on partitions 0:64, head B on 64:128)
  v_aug 16 seq-tiles [128, 4*65]: per head 64 v columns + a ones column
       (the ones column makes the oT matmul also produce the softmax
       denominator as row 64 of its output)
  sT   [k, q] scores transposed -> exp (no max subtraction: |s|~N(0,1), safe
       in fp32) -> pT
  oT   v_aug.T @ pT = [65, q]: rows 0:64 unnormalized head output (d on
       partitions), row 64 = softmax denominator

Out-projection (per block = one head-pair p, one 512-wide q chunk):
  reciprocal of the two denominator rows -> broadcast across 64 partitions
  with a K=33 selector matmul (R2) -> DVE-multiply into oT while casting to
  bf16 (oTs, normalized, heads A/B stacked on partitions) -> single K=128
  matmul per 128-q chunk against w2 covers both heads at once; p=0 result is
  copied to a SBUF accumulator, p=1 is added and DMA'd out. This replaces the
  K=64 matmul pairs + per-head tensor_scalar scaling + gpsimd adds of the
  earlier version (half the yproj PE rows, no transpose matmuls).
"""

import numpy as np

import concourse.bass as bass
import concourse.mybir as mybir
import concourse.tile as tile
from concourse import bacc

DIM = 512
NHEADS = 8
HD = 64
B = 4
SEQ = 2048
SCALE = HD ** -0.5

NCORES = 8
HPC = 4          # heads per core
QCH = 512        # q chunk (moving free dim)
NQC = SEQ // QCH # 4 q-chunks
KCH = 128        # k chunk (psum partition dim)
NKC = SEQ // KCH # 16 k-chunks
CCH = 128        # contraction chunk for projections
NCC = DIM // CCH # 4

F32 = mybir.dt.float32

BF16 = mybir.dt.bfloat16
# matmul input dtype. bf16: 1 cycle/row, FWL weight loads, half the PE power
# of f32r (less HAM throttling). fp8 was evaluated and fails the 2e-2
# correctness gate (rel err ~2.8e-2 in simulation).
MMDT = BF16


def _emit_o(nc, oA, oB, vaug_t, pt_pair, i, p, start, stop):
    """Accumulate the two kc chunks of pair-iteration i into oA/oB."""
    pA, pB = pt_pair
    for hh, (odst, psrc) in enumerate(((oA, pA), (oB, pB))):
        for half in range(2):
            kc = 2 * i + half
            nc.tensor.matmul(
                odst[:],
                vaug_t(kc)[:, 2 * p + hh, :],
                psrc[:, half * QCH:(half + 1) * QCH],
                start=(start and half == 0), stop=(stop and half == 1),
                skip_group_check=True,
            )


def build_nc():
    nc = bacc.Bacc()

    xT_d = nc.dram_tensor("xt", [DIM, SEQ], MMDT, kind="ExternalInput")
    wperm_d = nc.dram_tensor("wperm", [DIM, 4 * 128], MMDT, kind="ExternalInput")
    wv_d = nc.dram_tensor("wv", [DIM, HPC * HD], MMDT, kind="ExternalInput")
    w2_d = nc.dram_tensor("w2", [HPC * HD, DIM], MMDT, kind="ExternalInput")
    y_d = nc.dram_tensor("y", [SEQ, DIM], F32, kind="ExternalOutput")

    with tile.TileContext(nc) as tc:
        with (
            tc.tile_pool(name="const", bufs=1) as cpool,
            tc.tile_pool(name="big", bufs=1) as bigpool,
            tc.tile_pool(name="pt", bufs=4) as ptpool,
            tc.tile_pool(name="yacc", bufs=1) as yaccpool,
            tc.tile_pool(name="tmp", bufs=2) as tmppool,
            tc.tile_pool(name="small", bufs=2) as smallpool,
            tc.tile_pool(name="ps", bufs=1, space="PSUM") as ps,
        ):
            # ---- constants / inputs to SBUF ----
            xTt = cpool.tile([128, NCC * SEQ], MMDT, tag="xT", name="xT")
            wpt = cpool.tile([128, NCC * 512], MMDT, tag="wp", name="wp")
            wvt = cpool.tile([128, NCC * HPC * HD], MMDT, tag="wv", name="wv")
            w2t = cpool.tile([128, 2 * DIM], MMDT, tag="w2", name="w2")
            ones4 = cpool.tile([128, HPC], F32, tag="ones4")
            nc.gpsimd.memset(ones4[:], 1.0)
            ones1 = cpool.tile([1, 1], F32, tag="ones1")
            nc.gpsimd.memset(ones1[:], 1.0)
            # selector for the reciprocal broadcast: row 0 ones, rows 1:33
            # zero (K=33: K=1 matmuls fail an ISA check)
            selst = cpool.tile([33, 64], F32, tag="selst")
            nc.gpsimd.memset(selst[:], 0.0)
            nc.gpsimd.memset(selst[0:1, :], 1.0)
            sel64 = cpool.tile([33, 64], MMDT, tag="sel64")
            nc.vector.tensor_copy(sel64[:], selst[:])
            # preload the exp ACT table set early so the first real exp in
            # the attention phase doesn't stall the pipeline ~2.7us
            dummy = cpool.tile([1, 1], F32, tag="dummy")
            nc.scalar.activation(dummy[:], ones1[:],
                                 mybir.ActivationFunctionType.Exp)

            # three DMA-capable queues (sync/SP, scalar/ACT, gpsimd).
            # Plain 2D transfers (multi-dim interleaves start several us
            # late on hardware), ordered so the prelude's inputs (wperm
            # Q01/K01 cols, x cols 0:1024, wv) land first. Q23/K23 weight
            # cols and w2 are consumed tens of microseconds in.
            def xslice(c, a, b):
                return (xTt[:, c * SEQ + a:c * SEQ + b],
                        xT_d[c * 128:(c + 1) * 128, a:b])

            for c in range(NCC):   # Q01/K01 weight cols per c-chunk
                nc.sync.dma_start(wpt[:, c * 512:c * 512 + 256],
                                  wperm_d[c * 128:(c + 1) * 128, 0:256])
            for c in range(NCC):   # v weights per c-chunk
                nc.gpsimd.dma_start(
                    wvt[:, c * 256:(c + 1) * 256],
                    wv_d[c * 128:(c + 1) * 128, :])
            # x head halves, one per queue + tails balanced ~1MB/queue
            nc.scalar.dma_start(*xslice(0, 0, 1024))
            nc.sync.dma_start(*xslice(3, 0, 1024))
            nc.gpsimd.dma_start(*xslice(2, 0, 1024))
            nc.scalar.dma_start(*xslice(1, 0, 1024))
            nc.gpsimd.dma_start(*xslice(2, 1024, SEQ))
            nc.sync.dma_start(*xslice(3, 1024, SEQ))
            nc.scalar.dma_start(*xslice(1, 1024, SEQ))
            nc.scalar.dma_start(*xslice(0, 1024, SEQ))
            for c in range(NCC):   # Q23/K23 weight cols
                nc.sync.dma_start(wpt[:, c * 512 + 256:(c + 1) * 512],
                                  wperm_d[c * 128:(c + 1) * 128, 256:512])
            for g in range(2):
                nc.gpsimd.dma_start(w2t[:, g * DIM:(g + 1) * DIM],
                                    w2_d[g * 128:(g + 1) * 128, :])

            def xT_c(c):
                return xTt[:, c * SEQ:(c + 1) * SEQ]

            def wp_c(c):
                return wpt[:, c * 512:(c + 1) * 512]

            def wv_c(c):
                return wvt[:, c * (HPC * HD):(c + 1) * (HPC * HD)]

            def w2_p(p):
                return w2t[:, p * DIM:(p + 1) * DIM]

            # ---- persistent intermediates ----
            qkTs = [bigpool.tile([128, SEQ], MMDT, tag=f"qkT{m}",
                                 name=f"qkT{m}") for m in range(4)]
            vaugs = [bigpool.tile([128, HPC * 65], MMDT, tag=f"vaug{st}",
                                  name=f"vaug{st}") for st in range(NKC)]
            yacc = yaccpool.tile([128, SEQ // 128 * DIM], F32, tag="yacc")

            # zero-init the rcp33 pool bufs once; later writes touch row 0
            # only, so rows 1:33 stay zero for the K=33 broadcast matmul
            for _ in range(2):
                t = smallpool.tile([33, 2 * QCH], MMDT, tag="rcp33")
                nc.gpsimd.memset(t[:], 0.0)

            def qkT_blk(m):
                return qkTs[m]

            def vaug_t(kc):
                return vaugs[kc].rearrange("p (h e) -> p h e", e=65)

            def v_unit(st, tag, bufs):
                pv = ps.tile([128, HPC * HD], F32, tag=tag, bufs=bufs,
                             name="pv")
                for c in range(NCC):
                    nc.tensor.matmul(
                        pv[:],
                        xT_c(c)[:, st * 128:(st + 1) * 128],
                        wv_c(c)[:],
                        start=(c == 0), stop=(c == NCC - 1),
                        skip_group_check=True,
                    )
                vt = vaug_t(st)
                nc.vector.tensor_copy(
                    vt[:, :, 0:64], pv[:].rearrange("p (h d) -> p h d", d=HD)
                )
                nc.vector.tensor_copy(
                    vt[:, :, 64:65],
                    ones4[:].rearrange("p (h o) -> p h o", o=1))

            def qk_unit(m, s2, tag, bufs):
                pp = ps.tile([128, 512], F32, tag=tag, bufs=bufs, name="pp")
                for c in range(NCC):
                    nc.tensor.matmul(
                        pp[:],
                        wp_c(c)[:, m * 128:(m + 1) * 128],
                        xT_c(c)[:, s2 * 512:(s2 + 1) * 512],
                        start=(c == 0), stop=(c == NCC - 1),
                        skip_group_check=True,
                    )
                nc.vector.tensor_copy(qkTs[m][:, s2 * 512:(s2 + 1) * 512],
                                      pp[:])

            # ---- phase 1: minimal prelude, rest is in-block filler ----
            # The first attention block needs only Q01 for its q-chunk
            # (cols 0:512), K01 for its first k-chunks, and the first two
            # v seq-tiles; everything else is computed as filler inside
            # the attention blocks, scheduled against its first use.
            qk_unit(0, 0, "sA", 1)   # Q01 q 0:512
            v_unit(0, "sB", 1)
            qk_unit(1, 0, "y", 2)    # K01 k 0:512   (kc 0..3)
            v_unit(1, "sB", 1)
            v_unit(2, "y", 2)
            qk_unit(1, 1, "sA", 1)   # K01 k 512:1024 (kc 4..7)
            v_unit(3, "sB", 1)

            def V(st):
                return lambda: v_unit(st, "y", 2)

            def QK(m, s2):
                return lambda: qk_unit(m, s2, "y", 2)

            # filler schedule keyed by (p, qc, i). Deadlines: vaug(k) is
            # consumed by the o-emit at iteration k//2+1 of EVERY block (so
            # all v by end of block (0,0)); K01 s2=k at iteration 2k of
            # (0,0); Q01 s2=k at block (0,k); K23 all at (1,0); Q23 s2=k at
            # block (1,k).
            SCHED = {
                (0, 0, 0): [V(4), V(5)],
                (0, 0, 1): [V(6), QK(1, 2)],
                (0, 0, 2): [V(7), V(8)],
                (0, 0, 3): [V(9), QK(1, 3)],
                (0, 0, 4): [V(10), V(11)],
                (0, 0, 5): [V(12), V(13)],
                (0, 0, 6): [V(14), V(15), QK(0, 1)],
                (0, 1, 1): [QK(3, 0)],
                (0, 1, 3): [QK(0, 2)],
                (0, 1, 5): [QK(3, 1)],
                (0, 1, 7): [QK(3, 2)],
                (0, 2, 1): [QK(0, 3)],
                (0, 2, 3): [QK(3, 3)],
                (0, 2, 5): [QK(2, 0)],
                (0, 3, 1): [QK(2, 1)],
                (1, 0, 1): [QK(2, 2)],
                (1, 1, 1): [QK(2, 3)],
            }

            # ---- out-projection helpers ----
            def emit_release(p, qc, oA, oB):
                """Block epilogue: read oA/oB out quickly (reciprocal of the
                denominator rows + bf16 copy of the head outputs) so the
                next block's o-accumulation isn't WAR-stalled on them."""
                # approx reciprocal (~18 bits, denominators are 50..104 so
                # no edge cases) is ~5x faster than DVE reciprocal(); the
                # exact one (4us per call) stalled the whole release chain.
                # It needs an SBUF partition-0 source, hence the den
                # staging. Head A's chain runs fully before head B's so the
                # R2a matmul dependency resolves earliest; the otu copies
                # (the oA/oB WAR releases) interleave right behind.
                otu = tmppool.tile([128, QCH], MMDT, tag="otu")
                dens = smallpool.tile([1, 2 * QCH], F32, tag="dens")
                rcps = smallpool.tile([1, 2 * QCH], F32, tag="rcps")
                rcp33 = smallpool.tile([33, 2 * QCH], MMDT, tag="rcp33")
                nc.vector.tensor_copy(dens[:, 0:QCH], oA[64:65, :])
                nc.vector.reciprocal_approx_fast(rcps[:, 0:QCH],
                                                 dens[:, 0:QCH])
                nc.vector.tensor_copy(rcp33[0:1, 0:QCH], rcps[:, 0:QCH])
                nc.vector.tensor_copy(otu[0:64, :], oA[0:64, :])
                nc.vector.tensor_copy(dens[:, QCH:2 * QCH], oB[64:65, :])
                nc.vector.reciprocal_approx_fast(rcps[:, QCH:2 * QCH],
                                                 dens[:, QCH:2 * QCH])
                nc.vector.tensor_copy(rcp33[0:1, QCH:2 * QCH],
                                      rcps[:, QCH:2 * QCH])
                nc.vector.tensor_copy(otu[64:128, :], oB[0:64, :])
                return {"p": p, "qc": qc, "otu": otu, "rcp33": rcp33}

            def emit_R2_mults(pend):
                """Broadcast the reciprocals across 64 partitions (K=33
                selector matmuls) and scale otu into the normalized bf16
                stationary tile for the out-projection."""
                rcp33, otu = pend["rcp33"], pend["otu"]
                R2a = ps.tile([64, QCH], F32, tag="y", bufs=2, name="R2a")
                nc.tensor.matmul(R2a[:], sel64[:], rcp33[:, 0:QCH],
                                 start=True, stop=True, skip_group_check=True)
                R2b = ps.tile([64, QCH], F32, tag="y", bufs=2, name="R2b")
                nc.tensor.matmul(R2b[:], sel64[:], rcp33[:, QCH:2 * QCH],
                                 start=True, stop=True, skip_group_check=True)
                ot = tmppool.tile([128, QCH], MMDT, tag="oTs")
                nc.vector.tensor_mul(ot[0:64, :], otu[0:64, :], R2a[:])
                nc.vector.tensor_mul(ot[64:128, :], otu[64:128, :], R2b[:])
                pend["ot"] = ot

            def emit_yproj_j(pend, j):
                p, qc, ot = pend["p"], pend["qc"], pend["ot"]
                qt = qc * (QCH // 128) + j
                yps = ps.tile([128, DIM], F32, tag="y", bufs=2, name="yps")
                nc.tensor.matmul(
                    yps[:],
                    ot[:, j * 128:(j + 1) * 128],
                    w2_p(p)[:],
                    start=True, stop=True, skip_group_check=True,
                )
                ya = yacc[:, qt * DIM:(qt + 1) * DIM]
                if p == 0:
                    nc.vector.tensor_copy(ya, yps[:])
                else:
                    nc.vector.tensor_add(ya, ya, yps[:])
                    nc.sync.dma_start(y_d[qt * 128:(qt + 1) * 128, :], ya)

            # ---- phase 2: attention + out-proj ----
            # kc chunks processed in pairs: one s psum tile [128, 1024] holds
            # scores for kc and kc+1 side by side, halving ACT instruction
            # count. Two levels of software pipelining keep the PE stream
            # dense: within a block, s(i+1) is emitted before o(i) so the PE
            # never head-of-line blocks on exp(i); across blocks, the
            # out-projection of block n is spread into the first
            # pair-iterations of block n+1.
            NPAIR = NKC // 2

            pending = None
            for p in range(2):
                Q = qkT_blk(2 * p)
                K = qkT_blk(2 * p + 1)
                for qc in range(NQC):
                    oA = ps.tile([65, QCH], F32, tag="oA", bufs=1, name="oA")
                    oB = ps.tile([65, QCH], F32, tag="oB", bufs=1, name="oB")
                    # o-emission lags the scores by 1 pair-iteration, except
                    # that the first two pairs both emit at i=2: the first
                    # write to oA/oB then comes ~6.6us after the previous
                    # block ended, giving the release copies a wide margin
                    # (a WAR stall here demotes the PE clock for ~10us)
                    pend_o = []

                    def drain_o(upto, stop=False):
                        while pend_o and pend_o[0][0] <= upto:
                            j, pp_ = pend_o.pop(0)
                            _emit_o(nc, oA, oB, vaug_t, pp_, j, p,
                                    start=(j == 0),
                                    stop=(stop and j == NPAIR - 1))

                    for i in range(NPAIR):
                        sA = ps.tile([128, 2 * QCH], F32, tag="sA", bufs=1,
                                     name="sA")
                        sB = ps.tile([128, 2 * QCH], F32, tag="sB", bufs=1,
                                     name="sB")
                        for hh, stile in ((0, sA), (1, sB)):
                            for half in range(2):
                                kc = 2 * i + half
                                nc.tensor.matmul(
                                    stile[:, half * QCH:(half + 1) * QCH],
                                    K[64 * hh:64 * hh + 64,
                                      kc * 128:(kc + 1) * 128],
                                    Q[64 * hh:64 * hh + 64,
                                      qc * QCH:(qc + 1) * QCH],
                                    start=True, stop=True,
                                    skip_group_check=True,
                                )
                        for f in SCHED.get((p, qc, i), ()):
                            f()
                        # previous block's out-projection, spread across
                        # this block's pair-iterations: the R2/scale stage
                        # at i=2 (after enough PE work that the PE doesn't
                        # head-of-line block on the DVE chain), the four
                        # K=128 projection matmuls at i=4..7
                        if pending is not None:
                            if i == 3:
                                emit_R2_mults(pending)
                            elif 4 <= i < 4 + QCH // 128:
                                emit_yproj_j(pending, i - 4)
                                if i == 3 + QCH // 128:
                                    pending = None
                        if i >= 3:
                            drain_o(i - 1)
                        pA = ptpool.tile([128, 2 * QCH], MMDT, tag="pA")
                        pB = ptpool.tile([128, 2 * QCH], MMDT, tag="pB")
                        nc.scalar.activation(
                            pA[:], sA[:], mybir.ActivationFunctionType.Exp,
                            scale=SCALE,
                        )
                        nc.scalar.activation(
                            pB[:], sB[:], mybir.ActivationFunctionType.Exp,
                            scale=SCALE,
                        )
                        pend_o.append((i, (pA, pB)))
                    drain_o(NPAIR - 1, stop=True)
                    pending = emit_release(p, qc, oA, oB)

            # tail: last block's out-projection
            emit_R2_mults(pending)
            for j in range(QCH // 128):
                emit_yproj_j(pending, j)

    nc.finalize()
    return nc


_NC_CACHE = {}


def get_nc():
    if "nc" not in _NC_CACHE:
        _NC_CACHE["nc"] = build_nc()
    return _NC_CACHE["nc"]


def make_core_inputs(x, w_qkv, w_out):
    """Per-core input dicts (host-side sharding)."""
    in_maps = []
    for c in range(NCORES):
        b, hg = c // 2, c % 2
        heads = [hg * HPC + i for i in range(HPC)]
        qcols = [w_qkv[:, h * HD:(h + 1) * HD] for h in heads]
        kcols = [w_qkv[:, DIM + h * HD:DIM + (h + 1) * HD] for h in heads]
        vcols = [w_qkv[:, 2 * DIM + h * HD:2 * DIM + (h + 1) * HD] for h in heads]
        wperm = np.concatenate(
            [qcols[0], qcols[1], kcols[0], kcols[1],
             qcols[2], qcols[3], kcols[2], kcols[3]], axis=1)
        wv = np.concatenate(vcols, axis=1)
        w2 = w_out[hg * HPC * HD:(hg + 1) * HPC * HD, :]
        import ml_dtypes
        mmnp = (ml_dtypes.bfloat16 if MMDT == mybir.dt.bfloat16
                else np.float32)
        in_maps.append({
            "xt": np.ascontiguousarray(x[b].T).astype(mmnp),
            "wperm": np.ascontiguousarray(wperm).astype(mmnp),
            "wv": np.ascontiguousarray(wv).astype(mmnp),
            "w2": np.ascontiguousarray(w2).astype(mmnp),
        })
    return in_maps


def kernel(x, w_qkv, w_out, b_out):
    from concourse.bass_utils import run_bass_kernel_spmd

    x = np.asarray(x, dtype=np.float32)
    w_qkv = np.asarray(w_qkv, dtype=np.float32)
    w_out = np.asarray(w_out, dtype=np.float32)
    b_out = np.asarray(b_out, dtype=np.float32)

    nc = get_nc()
    in_maps = make_core_inputs(x, w_qkv, w_out)
    res = run_bass_kernel_spmd(nc, in_maps, list(range(NCORES))).results

    out = np.empty((B, SEQ, DIM), dtype=np.float32)
    for b in range(B):
        out[b] = res[2 * b]["y"] + res[2 * b + 1]["y"] + b_out
    return out


# revision 36
# speedup vs baseline: 1.0763x; 1.0234x over previous
"""Multi-head self-attention forward on 8 Trainium2 NeuronCores.

Problem: x[4,2048,512] -> qkv proj (w_qkv [512,1536]) -> 8-head attention
(head_dim 64) -> out proj (w_out [512,512] + b_out) -> y[4,2048,512].

Sharding: 8 shards = (batch b in 0..3) x (head-group hg in 0..1, 4 heads each).
Core c handles b=c//2, hg=c%2. Each core computes, for its batch and its 4
heads: qkv projection (only its heads' columns), attention, and the partial
output projection restricted to its heads' rows of w_out. Host sums the two
half-projections per batch and adds the bias.

On-device layout (all "T" tensors keep the contraction dim on partitions):
  xT   [512, 2048]   x[b] transposed (host-side transpose), one SBUF tile
       with the four 128-row chunks side by side in the free dim
  qkT  4 tiles [128, 2048]: Q01, K01, Q23, K23 (2 heads stacked per tile:
       head A on partitions 0:64, head B on 64:128)
  v_aug 16 seq-tiles [128, 4*65]: per head 64 v columns + a ones column
       (the ones column makes the oT matmul also produce the softmax
       denominator as row 64 of its output)
  sT   [k, q] scores transposed -> exp (no max subtraction: |s|~N(0,1), safe
       in fp32) -> pT
  oT   v_aug.T @ pT = [65, q]: rows 0:64 unnormalized head output (d on
       partitions), row 64 = softmax denominator

Out-projection (per block = one head-pair p, one 512-wide q chunk):
  reciprocal of the two denominator rows -> broadcast across 64 partitions
  with a K=33 selector matmul (R2) -> DVE-multiply into oT while casting to
  bf16 (oTs, normalized, heads A/B stacked on partitions) -> single K=128
  matmul per 128-q chunk against w2 covers both heads at once; p=0 result is
  copied to a SBUF accumulator, p=1 is added and DMA'd out. This replaces the
  K=64 matmul pairs + per-head tensor_scalar scaling + gpsimd adds of the
  earlier version (half the yproj PE rows, no transpose matmuls).
"""

import numpy as np

import concourse.bass as bass
import concourse.mybir as mybir
import concourse.tile as tile
from concourse import bacc

DIM = 512
NHEADS = 8
HD = 64
B = 4
SEQ = 2048
SCALE = HD ** -0.5

NCORES = 8
HPC = 4          # heads per core
QCH = 512        # q chunk (moving free dim)
NQC = SEQ // QCH # 4 q-chunks
KCH = 128        # k chunk (psum partition dim)
NKC = SEQ // KCH # 16 k-chunks
CCH = 128        # contraction chunk for projections
NCC = DIM // CCH # 4

F32 = mybir.dt.float32

BF16 = mybir.dt.bfloat16
# matmul input dtype. bf16: 1 cycle/row, FWL weight loads, half the PE power
# of f32r (less HAM throttling). fp8 was evaluated and fails the 2e-2
# correctness gate (rel err ~2.8e-2 in simulation).
MMDT = BF16


def _emit_o(nc, oA, oB, vaug_t, pt_pair, i, p, start, stop):
    """Accumulate the two kc chunks of pair-iteration i into oA/oB."""
    pA, pB = pt_pair
    for hh, (odst, psrc) in enumerate(((oA, pA), (oB, pB))):
        for half in range(2):
            kc = 2 * i + half
            nc.tensor.matmul(
                odst[:],
                vaug_t(kc)[:, 2 * p + hh, :],
                psrc[:, half * QCH:(half + 1) * QCH],
                start=(start and half == 0), stop=(stop and half == 1),
                skip_group_check=True,
            )


def build_nc():
    nc = bacc.Bacc()

    xT_d = nc.dram_tensor("xt", [DIM, SEQ], MMDT, kind="ExternalInput")
    wperm_d = nc.dram_tensor("wperm", [DIM, 4 * 128], MMDT, kind="ExternalInput")
    wv_d = nc.dram_tensor("wv", [DIM, HPC * HD], MMDT, kind="ExternalInput")
    w2_d = nc.dram_tensor("w2", [HPC * HD, DIM], MMDT, kind="ExternalInput")
    y_d = nc.dram_tensor("y", [SEQ, DIM], F32, kind="ExternalOutput")

    with tile.TileContext(nc) as tc:
        with (
            tc.tile_pool(name="const", bufs=1) as cpool,
            tc.tile_pool(name="big", bufs=1) as bigpool,
            tc.tile_pool(name="pt", bufs=4) as ptpool,
            tc.tile_pool(name="yacc", bufs=1) as yaccpool,
            tc.tile_pool(name="tmp", bufs=2) as tmppool,
            tc.tile_pool(name="small", bufs=2) as smallpool,
            tc.tile_pool(name="ps", bufs=1, space="PSUM") as ps,
        ):
            # ---- constants / inputs to SBUF ----
            xTt = cpool.tile([128, NCC * SEQ], MMDT, tag="xT", name="xT")
            wpt = cpool.tile([128, NCC * 512], MMDT, tag="wp", name="wp")
            wvt = cpool.tile([128, NCC * HPC * HD], MMDT, tag="wv", name="wv")
            w2t = cpool.tile([128, 2 * DIM], MMDT, tag="w2", name="w2")
            ones4 = cpool.tile([128, HPC], F32, tag="ones4")
            nc.gpsimd.memset(ones4[:], 1.0)
            ones1 = cpool.tile([1, 1], F32, tag="ones1")
            nc.gpsimd.memset(ones1[:], 1.0)
            # selector for the reciprocal broadcast: row 0 ones, rows 1:33
            # zero (K=33: K=1 matmuls fail an ISA check)
            selst = cpool.tile([33, 64], F32, tag="selst")
            nc.gpsimd.memset(selst[:], 0.0)
            nc.gpsimd.memset(selst[0:1, :], 1.0)
            sel64 = cpool.tile([33, 64], MMDT, tag="sel64")
            nc.vector.tensor_copy(sel64[:], selst[:])
            # preload the exp ACT table set early so the first real exp in
            # the attention phase doesn't stall the pipeline ~2.7us
            dummy = cpool.tile([1, 1], F32, tag="dummy")
            nc.scalar.activation(dummy[:], ones1[:],
                                 mybir.ActivationFunctionType.Exp)

            # three DMA-capable queues (sync/SP, scalar/ACT, gpsimd).
            # Plain 2D transfers (multi-dim interleaves start several us
            # late on hardware), ordered so the prelude's inputs (wperm
            # Q01/K01 cols, x cols 0:1024, wv) land first. Q23/K23 weight
            # cols and w2 are consumed tens of microseconds in.
            def xslice(c, a, b):
                return (xTt[:, c * SEQ + a:c * SEQ + b],
                        xT_d[c * 128:(c + 1) * 128, a:b])

            for c in range(NCC):   # Q01/K01 weight cols per c-chunk
                nc.sync.dma_start(wpt[:, c * 512:c * 512 + 256],
                                  wperm_d[c * 128:(c + 1) * 128, 0:256])
            for c in range(NCC):   # v weights per c-chunk
                nc.gpsimd.dma_start(
                    wvt[:, c * 256:(c + 1) * 256],
                    wv_d[c * 128:(c + 1) * 128, :])
            # x in priority column bands (prelude needs 0:512 of all four
            # c-chunks first, then 512:1024), tails balanced ~1MB/queue
            nc.scalar.dma_start(*xslice(0, 0, 512))
            nc.sync.dma_start(*xslice(3, 0, 512))
            nc.gpsimd.dma_start(*xslice(2, 0, 512))
            nc.scalar.dma_start(*xslice(1, 0, 512))
            nc.sync.dma_start(*xslice(3, 512, 1024))
            nc.gpsimd.dma_start(*xslice(2, 512, 1024))
            nc.scalar.dma_start(*xslice(0, 512, 1024))
            nc.scalar.dma_start(*xslice(1, 512, 1024))
            nc.gpsimd.dma_start(*xslice(2, 1024, SEQ))
            nc.sync.dma_start(*xslice(3, 1024, SEQ))
            nc.scalar.dma_start(*xslice(1, 1024, SEQ))
            nc.scalar.dma_start(*xslice(0, 1024, SEQ))
            for c in range(NCC):   # Q23/K23 weight cols
                nc.sync.dma_start(wpt[:, c * 512 + 256:(c + 1) * 512],
                                  wperm_d[c * 128:(c + 1) * 128, 256:512])
            for g in range(2):
                nc.gpsimd.dma_start(w2t[:, g * DIM:(g + 1) * DIM],
                                    w2_d[g * 128:(g + 1) * 128, :])

            def xT_c(c):
                return xTt[:, c * SEQ:(c + 1) * SEQ]

            def wp_c(c):
                return wpt[:, c * 512:(c + 1) * 512]

            def wv_c(c):
                return wvt[:, c * (HPC * HD):(c + 1) * (HPC * HD)]

            def w2_p(p):
                return w2t[:, p * DIM:(p + 1) * DIM]

            # ---- persistent intermediates ----
            qkTs = [bigpool.tile([128, SEQ], MMDT, tag=f"qkT{m}",
                                 name=f"qkT{m}") for m in range(4)]
            vaugs = [bigpool.tile([128, HPC * 65], MMDT, tag=f"vaug{st}",
                                  name=f"vaug{st}") for st in range(NKC)]
            yacc = yaccpool.tile([128, SEQ // 128 * DIM], F32, tag="yacc")

            # zero-init the rcp33 pool bufs once; later writes touch row 0
            # only, so rows 1:33 stay zero for the K=33 broadcast matmul
            for _ in range(2):
                t = smallpool.tile([33, 2 * QCH], MMDT, tag="rcp33")
                nc.gpsimd.memset(t[:], 0.0)

            def qkT_blk(m):
                return qkTs[m]

            def vaug_t(kc):
                return vaugs[kc].rearrange("p (h e) -> p h e", e=65)

            def v_unit(st, tag, bufs):
                pv = ps.tile([128, HPC * HD], F32, tag=tag, bufs=bufs,
                             name="pv")
                for c in range(NCC):
                    nc.tensor.matmul(
                        pv[:],
                        xT_c(c)[:, st * 128:(st + 1) * 128],
                        wv_c(c)[:],
                        start=(c == 0), stop=(c == NCC - 1),
                        skip_group_check=True,
                    )
                vt = vaug_t(st)
                nc.vector.tensor_copy(
                    vt[:, :, 0:64], pv[:].rearrange("p (h d) -> p h d", d=HD)
                )
                nc.vector.tensor_copy(
                    vt[:, :, 64:65],
                    ones4[:].rearrange("p (h o) -> p h o", o=1))

            def qk_unit(m, s2, tag, bufs):
                pp = ps.tile([128, 512], F32, tag=tag, bufs=bufs, name="pp")
                for c in range(NCC):
                    nc.tensor.matmul(
                        pp[:],
                        wp_c(c)[:, m * 128:(m + 1) * 128],
                        xT_c(c)[:, s2 * 512:(s2 + 1) * 512],
                        start=(c == 0), stop=(c == NCC - 1),
                        skip_group_check=True,
                    )
                nc.vector.tensor_copy(qkTs[m][:, s2 * 512:(s2 + 1) * 512],
                                      pp[:])

            # ---- phase 1: minimal prelude, rest is in-block filler ----
            # The first attention block needs only Q01 for its q-chunk
            # (cols 0:512), K01 for its first k-chunks, and the first two
            # v seq-tiles; everything else is computed as filler inside
            # the attention blocks, scheduled against its first use.
            qk_unit(0, 0, "sA", 1)   # Q01 q 0:512
            v_unit(0, "sB", 1)
            qk_unit(1, 0, "y", 2)    # K01 k 0:512   (kc 0..3)
            v_unit(1, "sB", 1)
            v_unit(2, "y", 2)
            qk_unit(1, 1, "sA", 1)   # K01 k 512:1024 (kc 4..7)
            v_unit(3, "sB", 1)

            def V(st):
                return lambda: v_unit(st, "y", 2)

            def QK(m, s2):
                return lambda: qk_unit(m, s2, "y", 2)

            # filler schedule keyed by (p, qc, i). Deadlines: vaug(k) is
            # consumed by the o-emit at iteration k//2+1 of EVERY block (so
            # all v by end of block (0,0)); K01 s2=k at iteration 2k of
            # (0,0); Q01 s2=k at block (0,k); K23 all at (1,0); Q23 s2=k at
            # block (1,k).
            SCHED = {
                (0, 0, 0): [V(4), V(5)],
                (0, 0, 1): [V(6), QK(1, 2)],
                (0, 0, 2): [V(7), V(8)],
                (0, 0, 3): [V(9), QK(1, 3)],
                (0, 0, 4): [V(10), V(11)],
                (0, 0, 5): [V(12), V(13)],
                (0, 0, 6): [V(14), V(15), QK(0, 1)],
                (0, 1, 1): [QK(3, 0)],
                (0, 1, 3): [QK(0, 2)],
                (0, 2, 1): [QK(0, 3)],
                (0, 2, 3): [QK(2, 0)],
                # the rest of Q23/K23 computes inside the p=1 blocks, where
                # the PE otherwise idles under the ACT wall (and any PE idle
                # gap risks a ~10us clock demotion); each chunk lands just
                # ahead of the k-sweep or q-chunk that consumes it
                (1, 0, 0): [QK(3, 1)],
                (1, 0, 1): [QK(3, 2)],
                (1, 0, 3): [QK(3, 3)],
                (1, 0, 5): [QK(2, 1)],
                (1, 1, 1): [QK(2, 2)],
                (1, 2, 1): [QK(2, 3)],
            }

            # ---- out-projection helpers ----
            def emit_release(p, qc, oA, oB):
                """Block epilogue: read oA/oB out quickly (reciprocal of the
                denominator rows + bf16 copy of the head outputs) so the
                next block's o-accumulation isn't WAR-stalled on them."""
                # approx reciprocal (~18 bits, denominators are 50..104 so
                # no edge cases) is ~5x faster than DVE reciprocal(); the
                # exact one (4us per call) stalled the whole release chain.
                # It needs an SBUF partition-0 source, hence the den
                # staging. Head A's chain runs fully before head B's so the
                # R2a matmul dependency resolves earliest; the otu copies
                # (the oA/oB WAR releases) interleave right behind.
                otu = tmppool.tile([128, QCH], MMDT, tag="otu")
                dens = smallpool.tile([1, 2 * QCH], F32, tag="dens")
                rcps = smallpool.tile([1, 2 * QCH], F32, tag="rcps")
                rcp33 = smallpool.tile([33, 2 * QCH], MMDT, tag="rcp33")
                nc.vector.tensor_copy(dens[:, 0:QCH], oA[64:65, :])
                nc.vector.reciprocal_approx_fast(rcps[:, 0:QCH],
                                                 dens[:, 0:QCH])
                nc.vector.tensor_copy(rcp33[0:1, 0:QCH], rcps[:, 0:QCH])
                nc.vector.tensor_copy(otu[0:64, :], oA[0:64, :])
                nc.vector.tensor_copy(dens[:, QCH:2 * QCH], oB[64:65, :])
                nc.vector.reciprocal_approx_fast(rcps[:, QCH:2 * QCH],
                                                 dens[:, QCH:2 * QCH])
                nc.vector.tensor_copy(rcp33[0:1, QCH:2 * QCH],
                                      rcps[:, QCH:2 * QCH])
                nc.vector.tensor_copy(otu[64:128, :], oB[0:64, :])
                return {"p": p, "qc": qc, "otu": otu, "rcp33": rcp33}

            def emit_R2_mults(pend):
                """Broadcast the reciprocals across 64 partitions (K=33
                selector matmuls) and scale otu into the normalized bf16
                stationary tile for the out-projection."""
                rcp33, otu = pend["rcp33"], pend["otu"]
                R2a = ps.tile([64, QCH], F32, tag="y", bufs=2, name="R2a")
                nc.tensor.matmul(R2a[:], sel64[:], rcp33[:, 0:QCH],
                                 start=True, stop=True, skip_group_check=True)
                R2b = ps.tile([64, QCH], F32, tag="y", bufs=2, name="R2b")
                nc.tensor.matmul(R2b[:], sel64[:], rcp33[:, QCH:2 * QCH],
                                 start=True, stop=True, skip_group_check=True)
                ot = tmppool.tile([128, QCH], MMDT, tag="oTs")
                nc.vector.tensor_mul(ot[0:64, :], otu[0:64, :], R2a[:])
                nc.vector.tensor_mul(ot[64:128, :], otu[64:128, :], R2b[:])
                pend["ot"] = ot

            def emit_yproj_j(pend, j):
                p, qc, ot = pend["p"], pend["qc"], pend["ot"]
                qt = qc * (QCH // 128) + j
                yps = ps.tile([128, DIM], F32, tag="y", bufs=2, name="yps")
                nc.tensor.matmul(
                    yps[:],
                    ot[:, j * 128:(j + 1) * 128],
                    w2_p(p)[:],
                    start=True, stop=True, skip_group_check=True,
                )
                ya = yacc[:, qt * DIM:(qt + 1) * DIM]
                if p == 0:
                    nc.vector.tensor_copy(ya, yps[:])
                else:
                    nc.vector.tensor_add(ya, ya, yps[:])
                    nc.sync.dma_start(y_d[qt * 128:(qt + 1) * 128, :], ya)

            # ---- phase 2: attention + out-proj ----
            # kc chunks processed in pairs: one s psum tile [128, 1024] holds
            # scores for kc and kc+1 side by side, halving ACT instruction
            # count. Two levels of software pipelining keep the PE stream
            # dense: within a block, s(i+1) is emitted before o(i) so the PE
            # never head-of-line blocks on exp(i); across blocks, the
            # out-projection of block n is spread into the first
            # pair-iterations of block n+1.
            NPAIR = NKC // 2

            pending = None
            for p in range(2):
                Q = qkT_blk(2 * p)
                K = qkT_blk(2 * p + 1)
                for qc in range(NQC):
                    oA = ps.tile([65, QCH], F32, tag="oA", bufs=1, name="oA")
                    oB = ps.tile([65, QCH], F32, tag="oB", bufs=1, name="oB")
                    # o-emission lags the scores by 1 pair-iteration, except
                    # that the first two pairs both emit at i=2: the first
                    # write to oA/oB then comes ~6.6us after the previous
                    # block ended, giving the release copies a wide margin
                    # (a WAR stall here demotes the PE clock for ~10us)
                    pend_o = []

                    def drain_o(upto, stop=False):
                        while pend_o and pend_o[0][0] <= upto:
                            j, pp_ = pend_o.pop(0)
                            _emit_o(nc, oA, oB, vaug_t, pp_, j, p,
                                    start=(j == 0),
                                    stop=(stop and j == NPAIR - 1))

                    for i in range(NPAIR):
                        sA = ps.tile([128, 2 * QCH], F32, tag="sA", bufs=1,
                                     name="sA")
                        sB = ps.tile([128, 2 * QCH], F32, tag="sB", bufs=1,
                                     name="sB")
                        for hh, stile in ((0, sA), (1, sB)):
                            for half in range(2):
                                kc = 2 * i + half
                                nc.tensor.matmul(
                                    stile[:, half * QCH:(half + 1) * QCH],
                                    K[64 * hh:64 * hh + 64,
                                      kc * 128:(kc + 1) * 128],
                                    Q[64 * hh:64 * hh + 64,
                                      qc * QCH:(qc + 1) * QCH],
                                    start=True, stop=True,
                                    skip_group_check=True,
                                )
                        for f in SCHED.get((p, qc, i), ()):
                            f()
                        # previous block's out-projection, spread across
                        # this block's pair-iterations: the R2/scale stage
                        # at i=2 (after enough PE work that the PE doesn't
                        # head-of-line block on the DVE chain), the four
                        # K=128 projection matmuls at i=4..7
                        if pending is not None:
                            if i == 3:
                                emit_R2_mults(pending)
                            elif 4 <= i < 4 + QCH // 128:
                                emit_yproj_j(pending, i - 4)
                                if i == 3 + QCH // 128:
                                    pending = None
                        if i >= 3:
                            drain_o(i - 1)
                        pA = ptpool.tile([128, 2 * QCH], MMDT, tag="pA")
                        pB = ptpool.tile([128, 2 * QCH], MMDT, tag="pB")
                        nc.scalar.activation(
                            pA[:], sA[:], mybir.ActivationFunctionType.Exp,
                            scale=SCALE,
                        )
                        nc.scalar.activation(
                            pB[:], sB[:], mybir.ActivationFunctionType.Exp,
                            scale=SCALE,
                        )
                        pend_o.append((i, (pA, pB)))
                    drain_o(NPAIR - 1, stop=True)
                    pending = emit_release(p, qc, oA, oB)

            # tail: last block's out-projection
            emit_R2_mults(pending)
            for j in range(QCH // 128):
                emit_yproj_j(pending, j)

    nc.finalize()
    return nc


_NC_CACHE = {}


def get_nc():
    if "nc" not in _NC_CACHE:
        _NC_CACHE["nc"] = build_nc()
    return _NC_CACHE["nc"]


def make_core_inputs(x, w_qkv, w_out):
    """Per-core input dicts (host-side sharding)."""
    in_maps = []
    for c in range(NCORES):
        b, hg = c // 2, c % 2
        heads = [hg * HPC + i for i in range(HPC)]
        qcols = [w_qkv[:, h * HD:(h + 1) * HD] for h in heads]
        kcols = [w_qkv[:, DIM + h * HD:DIM + (h + 1) * HD] for h in heads]
        vcols = [w_qkv[:, 2 * DIM + h * HD:2 * DIM + (h + 1) * HD] for h in heads]
        wperm = np.concatenate(
            [qcols[0], qcols[1], kcols[0], kcols[1],
             qcols[2], qcols[3], kcols[2], kcols[3]], axis=1)
        wv = np.concatenate(vcols, axis=1)
        w2 = w_out[hg * HPC * HD:(hg + 1) * HPC * HD, :]
        import ml_dtypes
        mmnp = (ml_dtypes.bfloat16 if MMDT == mybir.dt.bfloat16
                else np.float32)
        in_maps.append({
            "xt": np.ascontiguousarray(x[b].T).astype(mmnp),
            "wperm": np.ascontiguousarray(wperm).astype(mmnp),
            "wv": np.ascontiguousarray(wv).astype(mmnp),
            "w2": np.ascontiguousarray(w2).astype(mmnp),
        })
    return in_maps


def kernel(x, w_qkv, w_out, b_out):
    from concourse.bass_utils import run_bass_kernel_spmd

    x = np.asarray(x, dtype=np.float32)
    w_qkv = np.asarray(w_qkv, dtype=np.float32)
    w_out = np.asarray(w_out, dtype=np.float32)
    b_out = np.asarray(b_out, dtype=np.float32)

    nc = get_nc()
    in_maps = make_core_inputs(x, w_qkv, w_out)
    res = run_bass_kernel_spmd(nc, in_maps, list(range(NCORES))).results

    out = np.empty((B, SEQ, DIM), dtype=np.float32)
    for b in range(B):
        out[b] = res[2 * b]["y"] + res[2 * b + 1]["y"] + b_out
    return out
